# revision 4
# baseline (speedup 1.0000x reference)
"""GPT-2 (124M) forward pass on 8 Trainium2 NeuronCores.

Sharding: 4 pairs of cores; pair p owns sequence p (batch=4). Within a pair,
the 1024 tokens are split into eight 128-blocks, interleaved: even blocks on
the even core (parity 0), odd blocks on the odd core. Each core runs the
full-width transformer trunk on its 512 tokens; per layer one 2-rank
AllGather exchanges K^T and V between the pair. The LM head is computed
locally (own 512 tokens x full vocab); the host reassembles logits and
combines per-token sum-exp stats into the loss.

All matmuls run in bf16 with fp32 accumulation; LayerNorm, softmax and the
residual stream are fp32.
"""

from contextlib import ExitStack

import numpy as np
import ml_dtypes

import concourse.bass as bass
import concourse.bacc as bacc
import concourse.tile as tile
import concourse.mybir as mybir
from concourse.bass_utils import run_bass_kernel_spmd

BF16 = mybir.dt.bfloat16
F32 = mybir.dt.float32
I32 = mybir.dt.int32
AF = mybir.ActivationFunctionType
ALU = mybir.AluOpType

V, BLK, L, H, E = 50257, 1024, 12, 12, 768
D = 64                      # head dim
FF = 4 * E                  # 3072
P = 128                     # partitions
NB = 4                      # own token blocks per core
TL = NB * P                 # 512 local tokens
CT = E // P                 # 6 feature tiles
FT = FF // P                # 24 ff tiles
VCH = 512                   # vocab chunk
NVC = (V + VCH - 1) // VCH  # 99
VPAD = NVC * VCH            # 50688
N_CORES = 8
REPLICA_GROUPS = [[0, 1], [2, 3], [4, 5], [6, 7]]
NEG = -1.0e9
KT_SZ = E * TL
VA_SZ = TL * (H * 65)


def _ap(handle, offset, dims):
    return bass.AP(tensor=handle[:].tensor, offset=offset,
                   ap=[list(d) for d in dims])


def _pcol(handle, offset, n_tiles):
    """DRAM vector at offset -> sbuf [128, n_tiles] column-layout source AP."""
    return _ap(handle, offset, [(1, P), (P, n_tiles)])


def _bcast(handle, offset, width):
    """DRAM row [width] -> [128, width] partition-broadcast source AP."""
    return _ap(handle, offset, [(0, P), (1, width)])


def build(n_layers=L, debug=False):
    nc = bacc.Bacc("TRN2", target_bir_lowering=False, debug=False,
                   num_devices=N_CORES)

    t = {}
    t["idx"] = nc.dram_tensor("idx", [TL, 1], I32, kind="ExternalInput")
    t["wpe"] = nc.dram_tensor("wpe", [TL, E], F32, kind="ExternalInput")
    t["wte"] = nc.dram_tensor("wte", [V, E], F32, kind="ExternalInput")
    t["wteT"] = nc.dram_tensor("wteT", [E, VPAD], BF16, kind="ExternalInput")
    t["wqk"] = nc.dram_tensor("wqk", [L, E, 2 * E], BF16, kind="ExternalInput")
    t["wv"] = nc.dram_tensor("wv", [L, E, E], BF16, kind="ExternalInput")
    t["bqk"] = nc.dram_tensor("bqk", [L, 2 * E], F32, kind="ExternalInput")
    t["bv"] = nc.dram_tensor("bv", [L, E], F32, kind="ExternalInput")
    t["wap"] = nc.dram_tensor("wap", [L, E, E], BF16, kind="ExternalInput")
    t["bap"] = nc.dram_tensor("bap", [L, E], F32, kind="ExternalInput")
    t["wfc"] = nc.dram_tensor("wfc", [L, E, FF], BF16, kind="ExternalInput")
    t["bfc"] = nc.dram_tensor("bfc", [L, FF], F32, kind="ExternalInput")
    t["wmp"] = nc.dram_tensor("wmp", [L, FF, E], BF16, kind="ExternalInput")
    t["bmp"] = nc.dram_tensor("bmp", [L, E], F32, kind="ExternalInput")
    for nm in ("ln1w", "ln1b", "ln2w", "ln2b"):
        t[nm] = nc.dram_tensor(nm, [L, E], F32, kind="ExternalInput")
    t["lnfw"] = nc.dram_tensor("lnfw", [E], F32, kind="ExternalInput")
    t["lnfb"] = nc.dram_tensor("lnfb", [E], F32, kind="ExternalInput")
    t["maske"] = nc.dram_tensor("maske", [P, P], F32, kind="ExternalInput")
    t["masko"] = nc.dram_tensor("masko", [P, P], F32, kind="ExternalInput")
    t["ident"] = nc.dram_tensor("ident", [P, P], BF16, kind="ExternalInput")

    t["logits"] = nc.dram_tensor("logits", [TL, V], F32, kind="ExternalOutput")
    t["sumexp"] = nc.dram_tensor("sumexp", [TL, 1], F32, kind="ExternalOutput")
    if debug:
        t["xdbg"] = nc.dram_tensor("xdbg", [TL, E], F32, kind="ExternalOutput")

    t["kv_ins"] = [nc.dram_tensor(f"kvin{l}", [KT_SZ + VA_SZ], BF16)
                   for l in range(n_layers)]
    t["kv_outs"] = [nc.dram_tensor(f"kvout{l}", [2, KT_SZ + VA_SZ], BF16)
                    for l in range(n_layers)]

    with tile.TileContext(nc) as tc:
        _body(nc, tc, t, n_layers, debug)
    nc.compile()
    return nc


def _body(nc, tc, t, n_layers, debug):
    es = ExitStack()
    pers = es.enter_context(tc.tile_pool(name="pers", bufs=1))
    hpool = es.enter_context(tc.tile_pool(name="hpool", bufs=2))

    # ---- persistent tiles ----
    x_sb = pers.tile([P, NB, E], F32)
    vaug_own = pers.tile([P, NB, H, 65], BF16)
    ident_sb = pers.tile([P, P], BF16)
    maske_sb = pers.tile([P, P], F32)
    masko_sb = pers.tile([P, P], F32)
    eps_sb = pers.tile([P, 1], F32)
    ones_sb = pers.tile([1, D], F32)
    se_acc = pers.tile([P, NB], F32)

    nc.sync.dma_start(out=ident_sb[:], in_=t["ident"][:])
    nc.sync.dma_start(out=maske_sb[:], in_=t["maske"][:])
    nc.sync.dma_start(out=masko_sb[:], in_=t["masko"][:])
    nc.vector.memset(eps_sb[:], 1e-5)
    nc.vector.memset(ones_sb[:], 1.0)
    nc.vector.memset(vaug_own[:, :, :, 64:65], 1.0)
    nc.vector.memset(se_acc[:], 0.0)

    def layernorm_transpose(w_cols, b_cols, ps_pool, out_hT):
        """x_sb -> out_hT [128, CT, TL] bf16, per-feature w,b folded in."""
        for tb in range(NB):
            stats = hpool.tile([P, 3, 6], F32, tag="lnstats")
            for sg in range(3):
                nc.vector.bn_stats(out=stats[:, sg],
                                   in_=x_sb[:, tb, sg * 256:(sg + 1) * 256])
            mv = hpool.tile([P, 2], F32, tag="lnmv")
            nc.vector.bn_aggr(out=mv[:], in_=stats[:])
            rstd = hpool.tile([P, 1], F32, tag="lnrstd")
            nc.scalar.activation(out=rstd[:], in_=mv[:, 1:2], func=AF.Sqrt,
                                 bias=eps_sb[:], scale=1.0)
            nc.vector.reciprocal(out=rstd[:], in_=rstd[:])
            h_t = hpool.tile([P, E], BF16, tag="h")
            nc.vector.tensor_scalar(out=h_t[:], in0=x_sb[:, tb],
                                    scalar1=mv[:, 0:1], scalar2=rstd[:],
                                    op0=ALU.subtract, op1=ALU.mult)
            for ct in range(CT):
                tp = ps_pool.tile([P, P], BF16, tag="mm")
                nc.tensor.transpose(out=tp[:], in_=h_t[:, ct * P:(ct + 1) * P],
                                    identity=ident_sb[:])
                nc.vector.tensor_scalar(
                    out=out_hT[:, ct, tb * P:(tb + 1) * P], in0=tp[:],
                    scalar1=w_cols[:, ct:ct + 1], scalar2=b_cols[:, ct:ct + 1],
                    op0=ALU.mult, op1=ALU.add)

    # ---------------- embedding ----------------
    with tc.tile_pool(name="emb", bufs=2) as emb:
        idx_sb = emb.tile([P, NB], I32)
        nc.sync.dma_start(out=idx_sb[:], in_=_ap(t["idx"], 0, [(1, P), (P, NB)]))
        for tb in range(NB):
            g_t = emb.tile([P, E], F32, tag="gather")
            nc.gpsimd.indirect_dma_start(
                out=g_t[:], out_offset=None, in_=t["wte"][:],
                in_offset=bass.IndirectOffsetOnAxis(ap=idx_sb[:, tb:tb + 1], axis=0))
            p_t = emb.tile([P, E], F32, tag="wpe")
            nc.sync.dma_start(out=p_t[:], in_=t["wpe"][tb * P:(tb + 1) * P, :])
            nc.vector.tensor_add(out=x_sb[:, tb], in0=g_t[:], in1=p_t[:])

    # ================= transformer layers =================
    with ExitStack() as les:
        apool = les.enter_context(tc.tile_pool(name="apool", bufs=1))
        ppool = les.enter_context(tc.tile_pool(name="ppool", bufs=10))
        wpool = les.enter_context(tc.tile_pool(name="wpool", bufs=3))
        wmpool = les.enter_context(tc.tile_pool(name="wmpool", bufs=4))
        bpool = les.enter_context(tc.tile_pool(name="bpool", bufs=2))
        psA = les.enter_context(tc.tile_pool(name="psA", bufs=2, space="PSUM"))
        psW = les.enter_context(tc.tile_pool(name="psW", bufs=2, space="PSUM"))
        psV = les.enter_context(tc.tile_pool(name="psV", bufs=1, space="PSUM"))

        for l in range(n_layers):
            with nc.named_scope(f"layer{l}"):
                # ---- per-layer params ----
                ln1w = bpool.tile([P, CT], F32, tag="ln1w")
                ln1b = bpool.tile([P, CT], F32, tag="ln1b")
                ln2w = bpool.tile([P, CT], F32, tag="ln2w")
                ln2b = bpool.tile([P, CT], F32, tag="ln2b")
                nc.sync.dma_start(out=ln1w[:], in_=_pcol(t["ln1w"], l * E, CT))
                nc.sync.dma_start(out=ln1b[:], in_=_pcol(t["ln1b"], l * E, CT))
                nc.sync.dma_start(out=ln2w[:], in_=_pcol(t["ln2w"], l * E, CT))
                nc.sync.dma_start(out=ln2b[:], in_=_pcol(t["ln2b"], l * E, CT))
                bqk = bpool.tile([P, 2 * CT], F32, tag="bqk")
                nc.sync.dma_start(out=bqk[:], in_=_pcol(t["bqk"], l * 2 * E, 2 * CT))
                bfc = bpool.tile([P, FT], F32, tag="bfc")
                nc.sync.dma_start(out=bfc[:], in_=_pcol(t["bfc"], l * FF, FT))
                bv_b = bpool.tile([P, E], F32, tag="bv")
                nc.sync.dma_start(out=bv_b[:], in_=_bcast(t["bv"], l * E, E))
                bap_b = bpool.tile([P, E], F32, tag="bap")
                nc.sync.dma_start(out=bap_b[:], in_=_bcast(t["bap"], l * E, E))
                bmp_b = bpool.tile([P, E], F32, tag="bmp")
                nc.sync.dma_start(out=bmp_b[:], in_=_bcast(t["bmp"], l * E, E))

                # ---- ln1 + transpose ----
                hT = hpool.tile([P, CT, TL], BF16, tag="hT")
                layernorm_transpose(ln1w, ln1b, psA, hT)

                # ---- q^T,k^T feature-major ----
                qkT = apool.tile([P, 2 * CT, TL], BF16, tag="qkT")
                for half in range(2):
                    wslab = wpool.tile([P, CT, E], BF16, tag="w")
                    nc.sync.dma_start(out=wslab[:], in_=_ap(
                        t["wqk"], l * E * 2 * E + half * E,
                        [(2 * E, P), (P * 2 * E, CT), (1, E)]))
                    for ft in range(CT):
                        ps = psA.tile([P, TL], F32, tag="mm")
                        for ct in range(CT):
                            nc.tensor.matmul(ps[:],
                                             wslab[:, ct, ft * P:(ft + 1) * P],
                                             hT[:, ct, :],
                                             start=(ct == 0), stop=(ct == CT - 1))
                        fo = half * CT + ft
                        nc.vector.tensor_scalar(out=qkT[:, fo, :], in0=ps[:],
                                                scalar1=bqk[:, fo:fo + 1],
                                                scalar2=None, op0=ALU.add)

                # ---- v token-major into aug layout ----
                wslab = wpool.tile([P, CT, E], BF16, tag="w")
                nc.sync.dma_start(out=wslab[:], in_=_ap(
                    t["wv"], l * E * E, [(E, P), (P * E, CT), (1, E)]))
                for tb in range(NB):
                    ps = psW.tile([P, E], F32, tag="mmw")
                    for ct in range(CT):
                        lhs = hT[:, ct, tb * P:(tb + 1) * P]
                        nc.tensor.matmul(ps[:, 0:512], lhs, wslab[:, ct, 0:512],
                                         start=(ct == 0), stop=(ct == CT - 1))
                        nc.tensor.matmul(ps[:, 512:768], lhs, wslab[:, ct, 512:768],
                                         start=(ct == 0), stop=(ct == CT - 1))
                    nc.vector.tensor_tensor(
                        out=vaug_own[:, tb, :, 0:64],
                        in0=ps[:].rearrange("p (h d) -> p h d", h=H),
                        in1=bv_b[:].rearrange("p (h d) -> p h d", h=H),
                        op=ALU.add)

                # ---- AllGather K^T and V within the pair ----
                kv_in, kv_out = t["kv_ins"][l], t["kv_outs"][l]
                nc.gpsimd.dma_start(
                    out=_ap(kv_in, 0, [(TL, P), (P * TL, CT), (1, TL)]),
                    in_=qkT[:, CT:2 * CT, :])
                nc.gpsimd.dma_start(
                    out=_ap(kv_in, KT_SZ, [(H * 65, P), (P * H * 65, NB), (1, H * 65)]),
                    in_=vaug_own[:])
                nc.gpsimd.collective_compute(
                    "AllGather", ALU.bypass, ins=[kv_in[:]], outs=[kv_out[:]],
                    replica_groups=REPLICA_GROUPS)
                kT_all = apool.tile([P, CT, 2, TL], BF16, tag="kT_all")
                va_all = apool.tile([P, 2, NB, H, 65], BF16, tag="va_all")
                for r in range(2):
                    off = r * (KT_SZ + VA_SZ)
                    nc.gpsimd.dma_start(
                        out=kT_all[:, :, r, :],
                        in_=_ap(kv_out, off, [(TL, P), (P * TL, CT), (1, TL)]))
                    nc.gpsimd.dma_start(
                        out=va_all[:, r],
                        in_=_ap(kv_out, off + KT_SZ,
                                [(H * 65, P), (P * H * 65, NB), (1, H * 65)]))

                # ---- attention per head ----
                attT = apool.tile([P, CT, TL], BF16, tag="attT")
                for h in range(H):
                    po = (h % 2) * D
                    ft = h // 2
                    pubs = []
                    for g in range(8):
                        i0 = g // 2
                        n = (NB - i0) * P
                        r, j = g % 2, g // 2
                        ps = psA.tile([P, 512], F32, tag="mm")
                        nc.tensor.matmul(
                            ps[:, :n],
                            kT_all[po:po + D, ft, r, j * P:(j + 1) * P],
                            qkT[po:po + D, ft, i0 * P:TL],
                            start=True, stop=True)
                        msk = maske_sb if g % 2 == 0 else masko_sb
                        nc.vector.tensor_tensor(out=ps[:, 0:P], in0=ps[:, 0:P],
                                                in1=msk[:], op=ALU.add)
                        pu = ppool.tile([P, 512], BF16, tag="pu")
                        nc.scalar.activation(out=pu[:, :n], in_=ps[:, :n],
                                             func=AF.Exp, scale=float(D) ** -0.5)
                        pubs.append((pu, n, i0))
                    pa = psV.tile([65, 512], F32, tag="att")
                    for g in range(8):
                        pu, n, i0 = pubs[g]
                        r, j = g % 2, g // 2
                        nc.tensor.matmul(pa[:, i0 * P:TL],
                                         va_all[:, r, j, h, :],
                                         pu[:, :n], start=(g == 0), stop=(g == 7))
                    den = hpool.tile([1, TL], F32, tag="den")
                    nc.vector.tensor_copy(out=den[:], in_=pa[64:65, :])
                    pr = psV.tile([D, TL], F32, tag="rep")
                    nc.tensor.matmul(pr[:], ones_sb[:], den[:], start=True, stop=True)
                    rinv = hpool.tile([D, TL], F32, tag="rinv")
                    nc.vector.reciprocal(out=rinv[:], in_=pr[:])
                    nc.vector.tensor_tensor(out=attT[po:po + D, ft, :],
                                            in0=pa[0:D, :], in1=rinv[:],
                                            op=ALU.mult)

                # ---- attention out proj + residual ----
                wslab = wpool.tile([P, CT, E], BF16, tag="w")
                nc.sync.dma_start(out=wslab[:], in_=_ap(
                    t["wap"], l * E * E, [(E, P), (P * E, CT), (1, E)]))
                for tb in range(NB):
                    ps = psW.tile([P, E], F32, tag="mmw")
                    for at in range(CT):
                        lhs = attT[:, at, tb * P:(tb + 1) * P]
                        nc.tensor.matmul(ps[:, 0:512], lhs, wslab[:, at, 0:512],
                                         start=(at == 0), stop=(at == CT - 1))
                        nc.tensor.matmul(ps[:, 512:768], lhs, wslab[:, at, 512:768],
                                         start=(at == 0), stop=(at == CT - 1))
                    nc.vector.tensor_add(out=x_sb[:, tb], in0=x_sb[:, tb], in1=ps[:])
                    nc.vector.tensor_add(out=x_sb[:, tb], in0=x_sb[:, tb], in1=bap_b[:])

                # ---- ln2 + transpose ----
                h2T = hpool.tile([P, CT, TL], BF16, tag="hT")
                layernorm_transpose(ln2w, ln2b, psA, h2T)

                # ---- fc + gelu ----
                uT = apool.tile([P, FT, TL], BF16, tag="uT")
                for q in range(4):
                    wslab = wpool.tile([P, CT, E], BF16, tag="w")
                    nc.sync.dma_start(out=wslab[:], in_=_ap(
                        t["wfc"], l * E * FF + q * E,
                        [(FF, P), (P * FF, CT), (1, E)]))
                    for fl in range(CT):
                        ft = q * CT + fl
                        ps = psA.tile([P, TL], F32, tag="mm")
                        for ct in range(CT):
                            nc.tensor.matmul(ps[:],
                                             wslab[:, ct, fl * P:(fl + 1) * P],
                                             h2T[:, ct, :],
                                             start=(ct == 0), stop=(ct == CT - 1))
                        nc.scalar.activation(out=uT[:, ft, :], in_=ps[:],
                                             func=AF.Gelu, bias=bfc[:, ft:ft + 1],
                                             scale=1.0)

                # ---- mlp proj + residual ----
                wmps = []
                for q in range(4):
                    ws = wmpool.tile([P, CT, E], BF16, tag="wmp")
                    nc.sync.dma_start(out=ws[:], in_=_ap(
                        t["wmp"], l * FF * E + q * CT * P * E,
                        [(E, P), (P * E, CT), (1, E)]))
                    wmps.append(ws)
                for tb in range(NB):
                    ps = psW.tile([P, E], F32, tag="mmw")
                    for q in range(4):
                        for fl in range(CT):
                            ft = q * CT + fl
                            lhs = uT[:, ft, tb * P:(tb + 1) * P]
                            nc.tensor.matmul(ps[:, 0:512], lhs, wmps[q][:, fl, 0:512],
                                             start=(ft == 0), stop=(ft == FT - 1))
                            nc.tensor.matmul(ps[:, 512:768], lhs, wmps[q][:, fl, 512:768],
                                             start=(ft == 0), stop=(ft == FT - 1))
                    nc.vector.tensor_add(out=x_sb[:, tb], in0=x_sb[:, tb], in1=ps[:])
                    nc.vector.tensor_add(out=x_sb[:, tb], in0=x_sb[:, tb], in1=bmp_b[:])

    if debug:
        with tc.tile_pool(name="dbgp", bufs=2) as dbgp:
            for tb in range(NB):
                dt_ = dbgp.tile([P, E], F32, tag="dbg")
                nc.vector.tensor_copy(out=dt_[:], in_=x_sb[:, tb])
                nc.sync.dma_start(out=t["xdbg"][tb * P:(tb + 1) * P, :], in_=dt_[:])

    # ================= final ln + lm head =================
    with ExitStack() as hes:
        hfp = hes.enter_context(tc.tile_pool(name="hfp", bufs=1))
        hd = hes.enter_context(tc.tile_pool(name="hd", bufs=2))
        psH = hes.enter_context(tc.tile_pool(name="psH", bufs=6, space="PSUM"))
        psT = hes.enter_context(tc.tile_pool(name="psT", bufs=2, space="PSUM"))

        lnfw = hfp.tile([P, CT], F32)
        lnfb = hfp.tile([P, CT], F32)
        nc.sync.dma_start(out=lnfw[:], in_=_pcol(t["lnfw"], 0, CT))
        nc.sync.dma_start(out=lnfb[:], in_=_pcol(t["lnfb"], 0, CT))
        hfT = hfp.tile([P, CT, TL], BF16)
        layernorm_transpose(lnfw, lnfb, psT, hfT)

        VB = 6
        for tb in range(NB):
            for v0 in range(0, NVC, VB):
                vn = min(VB, NVC - v0)
                wt = hd.tile([P, CT, VB * VCH], BF16, tag="wteT")
                nc.sync.dma_start(out=wt[:, :, :vn * VCH], in_=_ap(
                    t["wteT"], v0 * VCH,
                    [(VPAD, P), (P * VPAD, CT), (1, vn * VCH)]))
                for vi in range(vn):
                    vb = v0 + vi
                    nvalid = min(VCH, V - vb * VCH)
                    ps = psH.tile([P, VCH], F32, tag="hmm")
                    for ct in range(CT):
                        nc.tensor.matmul(ps[:],
                                         hfT[:, ct, tb * P:(tb + 1) * P],
                                         wt[:, ct, vi * VCH:(vi + 1) * VCH],
                                         start=(ct == 0), stop=(ct == CT - 1))
                    lsb = hd.tile([P, VCH], F32, tag="lsb")
                    nc.vector.tensor_copy(out=lsb[:, :nvalid], in_=ps[:, :nvalid])
                    nc.sync.dma_start(
                        out=t["logits"][tb * P:(tb + 1) * P,
                                        vb * VCH:vb * VCH + nvalid],
                        in_=lsb[:, :nvalid])
                    ex = hd.tile([P, VCH], BF16, tag="ex")
                    sep = hd.tile([P, 1], F32, tag="sep")
                    nc.scalar.activation(out=ex[:, :nvalid], in_=ps[:, :nvalid],
                                         func=AF.Exp, accum_out=sep[:])
                    nc.vector.tensor_add(out=se_acc[:, tb:tb + 1],
                                         in0=se_acc[:, tb:tb + 1], in1=sep[:])
        nc.sync.dma_start(out=_ap(t["sumexp"], 0, [(1, P), (P, NB)]),
                          in_=se_acc[:])
    es.close()


# ======================= host side =======================

def host_prep(inputs):
    bf = ml_dtypes.bfloat16
    idx = np.asarray(inputs["idx"])
    wte = np.asarray(inputs["wte"], dtype=np.float32)
    wpe = np.asarray(inputs["wpe"], dtype=np.float32)
    wqkv = np.asarray(inputs["w_qkv"], dtype=np.float32)
    bqkv = np.asarray(inputs["b_qkv"], dtype=np.float32)

    wteT = np.zeros((E, VPAD), dtype=bf)
    wteT[:, :V] = wte.T.astype(bf)
    shared = {
        "wte": wte,
        "wteT": wteT,
        "wqk": np.ascontiguousarray(wqkv[:, :, :2 * E]).astype(bf),
        "wv": np.ascontiguousarray(wqkv[:, :, 2 * E:]).astype(bf),
        "bqk": np.ascontiguousarray(bqkv[:, :2 * E]),
        "bv": np.ascontiguousarray(bqkv[:, 2 * E:]),
        "wap": np.asarray(inputs["w_attn_proj"], np.float32).astype(bf),
        "bap": np.asarray(inputs["b_attn_proj"], np.float32),
        "wfc": np.asarray(inputs["w_fc"], np.float32).astype(bf),
        "bfc": np.asarray(inputs["b_fc"], np.float32),
        "wmp": np.asarray(inputs["w_mlp_proj"], np.float32).astype(bf),
        "bmp": np.asarray(inputs["b_mlp_proj"], np.float32),
        "ln1w": np.asarray(inputs["ln1_w"], np.float32),
        "ln1b": np.asarray(inputs["ln1_b"], np.float32),
        "ln2w": np.asarray(inputs["ln2_w"], np.float32),
        "ln2b": np.asarray(inputs["ln2_b"], np.float32),
        "lnfw": np.asarray(inputs["lnf_w"], np.float32),
        "lnfb": np.asarray(inputs["lnf_b"], np.float32),
        "ident": np.eye(P, dtype=bf),
    }
    # [s, t]: allow t >= s within the same global block
    diag = np.where(np.arange(P)[:, None] <= np.arange(P)[None, :],
                    0.0, NEG).astype(np.float32)
    in_maps = []
    for c in range(N_CORES):
        pair, par = c // 2, c % 2
        tok = own_tokens(par)
        m = dict(shared)
        m["idx"] = idx[pair][tok].astype(np.int32).reshape(TL, 1)
        m["wpe"] = wpe[tok]
        if par == 0:
            m["maske"] = diag
            m["masko"] = np.full((P, P), NEG, np.float32)
        else:
            m["maske"] = np.zeros((P, P), np.float32)
            m["masko"] = diag
        in_maps.append(m)
    return in_maps


def own_tokens(par):
    blocks = [par + 2 * j for j in range(NB)]
    return np.concatenate([np.arange(b * P, (b + 1) * P) for b in blocks])


def assemble(results, inputs):
    idx = np.asarray(inputs["idx"])
    targets = np.asarray(inputs["targets"])
    B, T = idx.shape
    logits = np.zeros((B, T, V), dtype=np.float32)
    sumexp = np.zeros((B, T), dtype=np.float64)
    for c in range(N_CORES):
        pair, par = c // 2, c % 2
        tok = own_tokens(par)
        logits[pair, tok] = results[c]["logits"]
        sumexp[pair, tok] = results[c]["sumexp"][:, 0].astype(np.float64)
    tgt = targets.reshape(-1)
    valid = tgt != -1
    flat = logits.reshape(-1, V)
    tgt_logit = flat[np.arange(flat.shape[0]), np.maximum(tgt, 0)].astype(np.float64)
    nll = np.log(sumexp.reshape(-1)) - tgt_logit
    loss = np.sum(np.where(valid, nll, 0.0)) / max(int(valid.sum()), 1)
    return logits, np.float32(loss)


_BUILT = {}


def kernel(**inputs):
    if "nc" not in _BUILT:
        _BUILT["nc"] = build()
    in_maps = host_prep(inputs)
    res = run_bass_kernel_spmd(_BUILT["nc"], in_maps, list(range(N_CORES)))
    return assemble(res.results, inputs)
